# revision 31
# baseline (speedup 1.0000x reference)
"""CenterLoss kernel for Trainium2, 8 NeuronCores, label-sorted sharding.

Math: the reference masks the full [B, C] squared-distance matrix with
one_hot(labels) and clamps to [1e-12, 1e12]; the kept entries are ~1024 so
the clamp only contributes the constant B*(C-1)*1e-12 and

    loss = ( sum_i ||x_i - c_{l_i}||^2 + B*(C-1)*1e-12 ) / B

Sharding (host): rows are sorted by label and split at class boundaries so
each core owns a contiguous range of <=128 classes and <=2176 rows (rows
zero-padded; pads excluded from the class counts so they contribute 0).
Per core, with s_k = sum_{i in k} x_i and q_k = sum_{i in k} x_i^2 (elemwise):

    loss_core = sum_{k,d} [ q - 2 c*s + n*c^2 ][k,d]

Both s and q come from one-hot matmuls on the (otherwise idle) tensor engine,
which kills the indirect-DMA gather of the data-parallel version entirely.
No tensor_tensor_reduce is used (it crashes the device on this runtime);
the only row reduction is one scalar-engine Copy+accum at the end.

  sync/scalar  : input DMAs spread over 2 HW queues (x in bf16)
  gpsimd : iota, even-chunk one-hots (is_equal), E_static = n*(cen*cen)
  vector : odd-chunk one-hots, x^2 for chunks 11-16, final combine
           t = E_static - 2*cen*S + Q   (PSUM reads via tensor_tensor only)
  tensor : 2 matmuls per chunk, accumulating S and Q PSUM banks
  scalar : x^2 for chunks 0-10 (Square activation), final row-reduce, store
Host sums the 8 x [128, 1] partials in f64 and adds the clamp constant.
"""

import sys
from contextlib import ExitStack

import numpy as np

try:
    import concourse.bass  # noqa: F401
except ImportError:
    sys.path.insert(0, "/opt/trn_rl_repo")

import concourse.bass as bass  # noqa: F401
import concourse.mybir as mybir
from concourse.bacc import Bacc
from concourse.bass_utils import run_bass_kernel_spmd

import ml_dtypes

B, C, D = 16384, 1000, 512
N_CORES = 8
P = 128
NCHUNK = 17  # row chunks per core
ROWCAP = P * NCHUNK  # 2176 >= max rows per core (~2054 for the target regime)
CLASSCAP = 128  # max classes per core (exactly 128 hit for the target regime)
CLAMP_MIN = 1e-12

# chunk -> x DMA group (4 groups: 4/4/4/5 chunks)
GROUPS = [(0, 4), (4, 8), (8, 12), (12, 17)]
SQ_SCALAR = list(range(0, 11))  # x^2 chunks on the scalar engine
SQ_VECTOR = list(range(11, 17))  # x^2 chunks on the vector engine

_NC_CACHE = {}


def build_nc():
    nc = Bacc()
    f32 = mybir.dt.float32
    bf16 = mybir.dt.bfloat16

    xb_d = nc.declare_dram_parameter("xb", [P, NCHUNK, D], bf16, isOutput=False)
    lbl_d = nc.declare_dram_parameter("llab", [P, NCHUNK], f32, isOutput=False)
    cen_d = nc.declare_dram_parameter("cen", [P, D], f32, isOutput=False)
    n_d = nc.declare_dram_parameter("nvec", [P, 1], f32, isOutput=False)
    out_d = nc.declare_dram_parameter("out", [P, 1], f32, isOutput=True)

    with ExitStack() as ctx:
        x_sb = ctx.enter_context(nc.sbuf_tensor("x_sb", [P, NCHUNK, D], bf16))
        xq_sb = ctx.enter_context(nc.sbuf_tensor("xq_sb", [P, NCHUNK, D], bf16))
        lbl_sb = ctx.enter_context(nc.sbuf_tensor("lbl_sb", [P, NCHUNK], f32))
        cen_sb = ctx.enter_context(nc.sbuf_tensor("cen_sb", [P, D], f32))
        n_sb = ctx.enter_context(nc.sbuf_tensor("n_sb", [P, 1], f32))
        iot_sb = ctx.enter_context(nc.sbuf_tensor("iot_sb", [P, P], f32))
        oh_sb = ctx.enter_context(nc.sbuf_tensor("oh_sb", [P, NCHUNK, P], bf16))
        c2_sb = ctx.enter_context(nc.sbuf_tensor("c2_sb", [P, D], f32))
        es_sb = ctx.enter_context(nc.sbuf_tensor("es_sb", [P, D], f32))
        t1_sb = ctx.enter_context(nc.sbuf_tensor("t1_sb", [P, D], f32))
        t2_sb = ctx.enter_context(nc.sbuf_tensor("t2_sb", [P, D], f32))
        t3_sb = ctx.enter_context(nc.sbuf_tensor("t3_sb", [P, D], f32))
        t4_sb = ctx.enter_context(nc.sbuf_tensor("t4_sb", [P, D], f32))
        junk_sb = ctx.enter_context(nc.sbuf_tensor("junk_sb", [P, D], f32))
        res_sb = ctx.enter_context(nc.sbuf_tensor("res_sb", [P, 1], f32))
        s_ps = ctx.enter_context(nc.psum_tensor("s_ps", [P, D], f32))
        q_ps = ctx.enter_context(nc.psum_tensor("q_ps", [P, D], f32))

        block = ctx.enter_context(nc.Block())
        ls = ctx.enter_context(nc.semaphore("ls"))
        xs = [ctx.enter_context(nc.semaphore(f"xs{g}")) for g in range(4)]
        cs = ctx.enter_context(nc.semaphore("cs"))
        ns2 = ctx.enter_context(nc.semaphore("ns2"))
        io = ctx.enter_context(nc.semaphore("io"))
        ohe = ctx.enter_context(nc.semaphore("ohe"))  # even one-hots (gpsimd)
        oho = ctx.enter_context(nc.semaphore("oho"))  # odd one-hots (vector)
        sqs = ctx.enter_context(nc.semaphore("sqs"))  # scalar x^2 count
        sqv = ctx.enter_context(nc.semaphore("sqv"))  # vector x^2 count
        ms = ctx.enter_context(nc.semaphore("ms"))  # matmul count (2/chunk)
        gs2 = ctx.enter_context(nc.semaphore("gs2"))  # E_static ready
        gch = ctx.enter_context(nc.semaphore("gch"))  # gpsimd RAW chain
        vch = ctx.enter_context(nc.semaphore("vch"))  # vector RAW chain
        cv = ctx.enter_context(nc.semaphore("cv"))  # t4 combine ready
        rr = ctx.enter_context(nc.semaphore("rr"))  # row-reduce done
        os_ = ctx.enter_context(nc.semaphore("os"))

        def grp(c):
            for g, (a, b) in enumerate(GROUPS):
                if a <= c < b:
                    return g
            raise AssertionError

        @block.sync
        def _(sync):
            sync.dma_start(out=lbl_sb[:], in_=lbl_d[:]).then_inc(ls, 16)
            for g in (0, 1):
                a, b = GROUPS[g]
                sync.dma_start(
                    out=x_sb[:, a:b, :], in_=xb_d[:, a:b, :]
                ).then_inc(xs[g], 16)
            sync.dma_start(out=n_sb[:], in_=n_d[:]).then_inc(ns2, 16)

        @block.tensor
        def _(tensor):
            for c in range(NCHUNK):
                if c % 2 == 0:
                    tensor.wait_ge(ohe, c // 2 + 1)
                else:
                    tensor.wait_ge(oho, (c - 1) // 2 + 1)
                tensor.wait_ge(xs[grp(c)], 16)
                tensor.matmul(
                    s_ps[:, :],
                    oh_sb[:, c, :],
                    x_sb[:, c, :],
                    start=(c == 0),
                    stop=(c == NCHUNK - 1),
                ).then_inc(ms, 1)
                if c in SQ_SCALAR:
                    tensor.wait_ge(sqs, SQ_SCALAR.index(c) + 1)
                else:
                    tensor.wait_ge(sqv, SQ_VECTOR.index(c) + 1)
                tensor.matmul(
                    q_ps[:, :],
                    oh_sb[:, c, :],
                    xq_sb[:, c, :],
                    start=(c == 0),
                    stop=(c == NCHUNK - 1),
                ).then_inc(ms, 1)

        @block.gpsimd
        def _(gpsimd):
            gpsimd.iota(
                iot_sb[:, :],
                [[1, P]],
                channel_multiplier=0,
                allow_small_or_imprecise_dtypes=True,
            ).then_inc(io, 1)
            gpsimd.wait_ge(io, 1)  # iota completes async even on its own engine
            gpsimd.wait_ge(ls, 16)
            for c in range(0, NCHUNK, 2):
                gpsimd.tensor_scalar(
                    oh_sb[:, c, :],
                    iot_sb[:, :],
                    lbl_sb[:, c : c + 1],
                    None,
                    mybir.AluOpType.is_equal,
                ).then_inc(ohe, 1)
            # E_static = n * cen^2, off the critical path
            gpsimd.wait_ge(cs, 16)
            gpsimd.tensor_tensor(
                out=c2_sb[:, :],
                in0=cen_sb[:, :],
                in1=cen_sb[:, :],
                op=mybir.AluOpType.mult,
            ).then_inc(gch, 1)
            gpsimd.wait_ge(gch, 1)
            gpsimd.wait_ge(ns2, 16)
            gpsimd.tensor_scalar(
                es_sb[:, :],
                c2_sb[:, :],
                n_sb[:, 0:1],
                None,
                mybir.AluOpType.mult,
            ).then_inc(gs2, 1)

        @block.vector
        def _(vector):
            vector.wait_ge(ls, 16)
            vector.wait_ge(io, 1)
            for c in range(1, NCHUNK, 2):
                vector.tensor_scalar(
                    oh_sb[:, c, :],
                    iot_sb[:, :],
                    lbl_sb[:, c : c + 1],
                    None,
                    mybir.AluOpType.is_equal,
                ).then_inc(oho, 1)
            for c in SQ_VECTOR:
                vector.wait_ge(xs[grp(c)], 16)
                vector.tensor_tensor(
                    out=xq_sb[:, c, :],
                    in0=x_sb[:, c, :],
                    in1=x_sb[:, c, :],
                    op=mybir.AluOpType.mult,
                ).then_inc(sqv, 1)
            # final combine: t4 = es - 2*cen*S + Q
            vector.wait_ge(ms, 2 * NCHUNK)
            vector.wait_ge(cs, 16)
            vector.tensor_tensor(
                out=t1_sb[:, :],
                in0=cen_sb[:, :],
                in1=s_ps[:, :],
                op=mybir.AluOpType.mult,
            ).then_inc(vch, 1)
            vector.wait_ge(gs2, 1)
            vector.wait_ge(vch, 1)
            vector.tensor_scalar(
                t2_sb[:, :],
                t1_sb[:, :],
                -2.0,
                None,
                mybir.AluOpType.mult,
            ).then_inc(vch, 1)
            vector.wait_ge(vch, 2)
            vector.tensor_tensor(
                out=t3_sb[:, :],
                in0=t2_sb[:, :],
                in1=es_sb[:, :],
                op=mybir.AluOpType.add,
            ).then_inc(vch, 1)
            vector.wait_ge(vch, 3)
            vector.tensor_tensor(
                out=t4_sb[:, :],
                in0=t3_sb[:, :],
                in1=q_ps[:, :],
                op=mybir.AluOpType.add,
            ).then_inc(cv, 1)

        @block.scalar
        def _(scalar):
            for g in (2, 3):
                a, b = GROUPS[g]
                scalar.dma_start(
                    out=x_sb[:, a:b, :], in_=xb_d[:, a:b, :]
                ).then_inc(xs[g], 16)
            scalar.dma_start(out=cen_sb[:], in_=cen_d[:]).then_inc(cs, 16)
            for c in SQ_SCALAR:
                scalar.wait_ge(xs[grp(c)], 16)
                scalar.activation(
                    out=xq_sb[:, c, :],
                    in_=x_sb[:, c, :],
                    func=mybir.ActivationFunctionType.Square,
                ).then_inc(sqs, 1)
            scalar.wait_ge(cv, 1)
            scalar.activation(
                out=junk_sb[:, :],
                in_=t4_sb[:, :],
                func=mybir.ActivationFunctionType.Copy,
                accum_out=res_sb[:, 0:1],
            ).then_inc(rr, 1)
            scalar.wait_ge(rr, 1)
            scalar.dma_start(out=out_d[:], in_=res_sb[:]).then_inc(os_, 16)
            scalar.wait_ge(os_, 16)

    nc.finalize()
    return nc


def _get_nc():
    if "nc" not in _NC_CACHE:
        _NC_CACHE["nc"] = build_nc()
    return _NC_CACHE["nc"]


def _core_bounds(cnt):
    """Class-aligned boundaries splitting rows ~evenly across 8 cores."""
    cum = np.concatenate([[0], np.cumsum(cnt)])
    bounds = [0]
    for j in range(1, N_CORES):
        bounds.append(int(np.searchsorted(cum, j * (B // N_CORES), side="left")))
    bounds.append(C)
    for j in range(N_CORES):
        rows = int(cum[bounds[j + 1]] - cum[bounds[j]])
        ncls = bounds[j + 1] - bounds[j]
        assert rows <= ROWCAP, (j, rows)
        assert ncls <= CLASSCAP, (j, ncls)
    return bounds, cum


def kernel(x, labels, centers, _trace=False):
    x = np.asarray(x, dtype=np.float32)
    centers = np.asarray(centers, dtype=np.float32)
    labels_i = np.asarray(labels).astype(np.int64)

    cnt = np.bincount(labels_i, minlength=C)
    bounds, cum = _core_bounds(cnt)
    order = np.argsort(labels_i, kind="stable")
    x_sorted = x[order]
    lab_sorted = labels_i[order]

    in_maps = []
    for j in range(N_CORES):
        lo, hi = bounds[j], bounds[j + 1]
        r0, r1 = int(cum[lo]), int(cum[hi])
        rows = r1 - r0
        ncls = hi - lo

        xb = np.zeros((ROWCAP, D), dtype=ml_dtypes.bfloat16)
        xb[:rows] = x_sorted[r0:r1]
        xb = np.ascontiguousarray(
            xb.reshape(NCHUNK, P, D).transpose(1, 0, 2)
        )  # [128, 17, 512], row 128c+p at [p, c]

        llab = np.zeros(ROWCAP, dtype=np.float32)
        llab[:rows] = (lab_sorted[r0:r1] - lo).astype(np.float32)
        llab = np.ascontiguousarray(llab.reshape(NCHUNK, P).T)  # [128, 17]

        cen = np.zeros((P, D), dtype=np.float32)
        cen[:ncls] = centers[lo:hi]

        nvec = np.zeros((P, 1), dtype=np.float32)
        nvec[:ncls, 0] = cnt[lo:hi]

        in_maps.append({"xb": xb, "llab": llab, "cen": cen, "nvec": nvec})

    nc = _get_nc()
    res = run_bass_kernel_spmd(nc, in_maps, list(range(N_CORES)), trace=_trace)
    parts = np.stack([r["out"] for r in res.results]).astype(np.float64)  # [8,128,1]
    total = parts.sum() + B * (C - 1) * CLAMP_MIN
    loss = np.float32(total / B)
    if _trace:
        return np.asarray(loss), res
    return np.asarray(loss)
